# revision 15
# baseline (speedup 1.0000x reference)
"""GPPT (GCN + prompt MoE routing) Trainium2 kernel, 8-core SPMD.

Row-shards the N=8192 nodes across 8 NeuronCores (1024 rows each).
Each core holds its block of adj (pre-transposed + scaled on host) and
computes:

  L0:    T^T  = feature^T @ adjT_blk          (3-pass fp16 hi/lo split)
  h0^T   = relu((W0^T @ T^T) * 2^-13 + b0)    (f32r)
  Y1_blk = h0_blk @ (W1*8192)                 (f32r) -> fp16 hi/lo
  4 chunked AllGathers of Y1 (hi/lo x node-halves), first triggered
    before the Y1 phase finishes; expert h0-half precomputed in the
    gather latency window.
  L1:    h1^T = relu((Y1^T @ adjT_blk) * 2^-26 + b1), restructured as
    4 sweeps (hi-half0, hi-half1, lo-half0, lo-half1) so each sweep
    only waits on its own chunked collective.  Collective-dependent
    Y DMAs issue on the ACT hwdge queues so they can't head-of-line
    block the adj stream on the SP queues.
  scores/experts: hc @ [Wp | WppT]            (f32r), one-hot select

Matmuls are grouped by stationary operand (one fp16 lhsT slice feeds
2-4 consecutive matmuls) to enable LDWEIGHTS amortization.

Precision: the expert routing argmax has a 2.5e-7 minimum top-2 score
gap on this input; the two big adj matmuls therefore run as 3-pass
fp16 hi/lo split products (A~Ah+Al, X~Xh+Xl, AX~AhXh+AlXh+AhXl) with
adj scaled by 8192 so the fp16 splits stay in the normal range. This
is fp32-grade (verified: 0 routing flips, ~5e-7 output rel err).
"""

import os
import numpy as np

import concourse.bass as bass
import concourse.bass_utils as _bass_utils
import concourse.mybir as mybir
import concourse.tile as tile
from concourse import bacc
from concourse.bass_utils import run_bass_kernel_spmd


N = 8192
IN = 512
H = 512
C = 32
E = 7
NCORES = 8
BLK = N // NCORES          # 1024 nodes per core
KT = N // 128              # 64 contraction k-tiles over nodes
SCALE = 8192.0             # adj pre-scale (exact power of two)
OC = E * C                 # 224 expert-head output cols (fp16 matmul)
SCOL = 256                 # psum col offset of the fp32 score region
TW = SCOL + E + 1          # 264 cols drained from psum (oa | pad | scores)

F32 = mybir.dt.float32
F16 = mybir.dt.float16

# stashed by kernel() for test harnesses: BassKernelResults of last run
LAST_RESULTS = None
_CACHED_NC = None


def _kernel_body(ctx, tc, aps):
    nc = tc.nc
    AFT = mybir.ActivationFunctionType
    ALU = mybir.AluOpType

    A_h, A_l = aps["A_h"], aps["A_l"]
    F_h, F_l = aps["F_h"], aps["F_l"]
    W0, W1s = aps["W0"], aps["W1s"]
    b0, b1 = aps["b0"], aps["b1"]
    Wcat16 = aps["Wcat16"]      # [2H, 224] = WppT in fp16 (expert heads)
    Wp32 = aps["Wp32"]          # [2H, 7] fp32 prompt/routing weights
    iota7 = aps["iota7"]        # [128, 7] fp32 0..6 per row
    out = aps["out"]
    cch_in = [aps["cc_h0_in"], aps["cc_h1_in"]]
    ccl_in = [aps["cc_l0_in"], aps["cc_l1_in"]]
    cch_out = [aps["cc_h0_out"], aps["cc_h1_out"]]
    ccl_out = [aps["cc_l0_out"], aps["cc_l1_out"]]

    const = ctx.enter_context(tc.tile_pool(name="const", bufs=1))
    acts = ctx.enter_context(tc.tile_pool(name="acts", bufs=1))
    stream = ctx.enter_context(tc.tile_pool(name="stream", bufs=4))
    ypool = ctx.enter_context(tc.tile_pool(name="ypool", bufs=3))
    yswh = ctx.enter_context(tc.tile_pool(name="yswh", bufs=24))
    yswl = ctx.enter_context(tc.tile_pool(name="yswl", bufs=12))
    astream = ctx.enter_context(tc.tile_pool(name="astream", bufs=8))
    small = ctx.enter_context(tc.tile_pool(name="small", bufs=4))
    psum = ctx.enter_context(tc.tile_pool(name="psum", bufs=1, space="PSUM"))

    # ---- 8 PSUM bank accumulators, reused phase to phase ----
    ps = [psum.tile([128, 512], F32, name=f"bank{i}") for i in range(8)]

    # ---- L0 stream tiles ----
    def l0_tiles(k):
        fh = stream.tile([128, IN], F16, name="fh")
        fl = stream.tile([128, IN], F16, name="fl")
        ah = stream.tile([128, BLK], F16, name="ah")
        al = stream.tile([128, BLK], F16, name="al")
        r = slice(k * 128, (k + 1) * 128)
        nc.sync.dma_start(fh[:], F_h[r, :])
        nc.sync.dma_start(fl[:], F_l[r, :])
        nc.sync.dma_start(ah[:, 0:512], A_h[r, 0:512])
        nc.sync.dma_start(ah[:, 512:1024], A_h[r, 512:1024])
        nc.sync.dma_start(al[:, 0:512], A_l[r, 0:512])
        nc.sync.dma_start(al[:, 512:1024], A_l[r, 512:1024])
        return fh, fl, ah, al

    # =========== L0: TT[m,n] = sum_k F[k][:,m].T @ A[k][:,n] (3-pass) =====
    # grouped by stationary: fh_m feeds 4 matmuls, fl_m feeds 2.
    for k in range(KT):
        fh, fl, ah, al = l0_tiles(k)
        for m in range(4):
            lt = fh[:, m * 128:(m + 1) * 128]
            for idx, (rt, n) in enumerate(((ah, 0), (ah, 1), (al, 0), (al, 1))):
                nc.tensor.matmul(
                    ps[m * 2 + n][:], lt, rt[:, n * 512:(n + 1) * 512],
                    start=(k == 0 and idx < 2), stop=False,
                )
        for m in range(4):
            lt = fl[:, m * 128:(m + 1) * 128]
            for n in range(2):
                nc.tensor.matmul(
                    ps[m * 2 + n][:], lt, ah[:, n * 512:(n + 1) * 512],
                    start=False, stop=(k == KT - 1),
                )

    # ---- constants / weights resident in SBUF.  Emitted after the L0
    # loop so they queue behind the first adj k-tiles (startup latency),
    # but arrive long before their first use. ----
    w0_t = []
    w1_t = []
    for k in range(4):
        t = const.tile([128, H], F32, name=f"w0_{k}")
        nc.sync.dma_start(t[:], W0[k * 128:(k + 1) * 128, :])
        w0_t.append(t)
        t = const.tile([128, H], F32, name=f"w1_{k}")
        nc.sync.dma_start(t[:], W1s[k * 128:(k + 1) * 128, :])
        w1_t.append(t)
    wcat_t = []
    wp_t = []
    for k in range(8):
        t = const.tile([128, OC], F16, name=f"wcat_{k}")
        nc.sync.dma_start(t[:], Wcat16[k * 128:(k + 1) * 128, :])
        wcat_t.append(t)
        t = const.tile([128, E], F32, name=f"wp_{k}")
        nc.sync.dma_start(t[:], Wp32[k * 128:(k + 1) * 128, :])
        wp_t.append(t)
    b0_t = []
    b1_t = []
    for m in range(4):
        t = const.tile([128, 1], F32, name=f"b0_{m}")
        nc.sync.dma_start(t[:], b0[m * 128:(m + 1) * 128, :])
        b0_t.append(t)
        t = const.tile([128, 1], F32, name=f"b1_{m}")
        nc.sync.dma_start(t[:], b1[m * 128:(m + 1) * 128, :])
        b1_t.append(t)
    iota_t = const.tile([128, E], F32, name="iota7")
    nc.sync.dma_start(iota_t[:], iota7[:, :])

    # copy TT out of PSUM (raw, still scaled by 8192); node-half n=0
    # first so the first half of the W0 apply can start sooner.
    tt = [acts.tile([128, BLK], F32, name=f"tt_{m}") for m in range(4)]
    for n in range(2):
        for m in range(4):
            nc.vector.tensor_copy(tt[m][:, n * 512:(n + 1) * 512], ps[m * 2 + n][:])

    # h0T[m,n] = relu(2^-13 * sum_k W0[k][:,m].T @ TT[k][:,n] + b0)
    # n-major: the n=0 node-half feeds Y1 chunks 0..3 (= the first
    # AllGather payload), so it runs before the n=1 half.
    h0t = [acts.tile([128, BLK], F32, name=f"h0t_{m}") for m in range(4)]
    h0t16 = [acts.tile([128, BLK], F16, name=f"h0t16_{m}") for m in range(4)]

    def w0_apply(n):
        for m in range(4):
            pt = ps[m * 2 + n]
            for k in range(4):
                nc.tensor.matmul(
                    pt[:],
                    w0_t[k][:, m * 128:(m + 1) * 128],
                    tt[k][:, n * 512:(n + 1) * 512],
                    start=(k == 0),
                    stop=(k == 3),
                )
            nc.scalar.activation(
                h0t[m][:, n * 512:(n + 1) * 512], pt[:],
                AFT.Relu, bias=b0_t[m][:], scale=1.0 / SCALE,
            )
            nc.vector.tensor_copy(
                h0t16[m][:, n * 512:(n + 1) * 512],
                h0t[m][:, n * 512:(n + 1) * 512])

    # =========== Y1s[m] = sum_k h0t[k][:,m].T @ W1s[k]  (node-major) =======
    # fp16 hi/lo to 4 chunked cc buffers; first AllGather fires at m==3.
    def ag(ib, ob):
        nc.gpsimd.collective_compute(
            "AllGather",
            mybir.AluOpType.bypass,
            replica_groups=[list(range(NCORES))],
            ins=[ib.opt()],
            outs=[ob.opt()],
        )

    def y1_chunk(m):
        pt = ps[m]
        for k in range(4):
            nc.tensor.matmul(
                pt[:],
                h0t[k][:, m * 128:(m + 1) * 128],
                w1_t[k][:],
                start=(k == 0),
                stop=(k == 3),
            )
        yh = ypool.tile([128, H], F16, name="yh")
        yl = ypool.tile([128, H], F16, name="yl")
        nc.vector.tensor_copy(yh[:], pt[:])
        nc.vector.tensor_tensor(yl[:], pt[:], yh[:], op=mybir.AluOpType.subtract)
        half, row = divmod(m * 128, 512)
        nc.sync.dma_start(cch_in[half][row:row + 128, :], yh[:])
        nc.sync.dma_start(ccl_in[half][row:row + 128, :], yl[:])

    w0_apply(0)
    for m in range(4):
        y1_chunk(m)
    ag(cch_in[0], cch_out[0])
    w0_apply(1)
    for m in range(4, 8):
        y1_chunk(m)
    ag(cch_in[1], cch_out[1])
    ag(ccl_in[0], ccl_out[0])
    ag(ccl_in[1], ccl_out[1])

    # ---- expert h0-half precompute (fills part of the gather latency) ----
    # oa0[m] = sum_{k=4..7} hc16[k] @ Wcat16[k] (fp16 out_all half)
    # sc0[m] = sum_{k=4..7} hc[k] @ Wp32[k]     (fp32 score half)
    oa0 = [acts.tile([128, OC], F16, name=f"oa0_{m}") for m in range(8)]
    sc0 = [acts.tile([128, E], F32, name=f"sc0_{m}") for m in range(8)]
    for m in range(8):
        pt = ps[m]
        for k in range(4):
            nc.tensor.matmul(
                pt[:, 0:OC],
                h0t16[k][:, m * 128:(m + 1) * 128],
                wcat_t[4 + k][:],
                start=(k == 0),
                stop=(k == 3),
            )
        for k in range(4):
            nc.tensor.matmul(
                pt[:, SCOL:SCOL + E],
                h0t[k][:, m * 128:(m + 1) * 128],
                wp_t[4 + k][:],
                start=(k == 0),
                stop=(k == 3),
            )
        nc.vector.tensor_copy(oa0[m][:], pt[:, 0:OC])
        nc.vector.tensor_copy(sc0[m][:], pt[:, SCOL:SCOL + E])

    # =========== L1: h1T = sum over nodes of Y^T @ adjT (3-pass) ==========
    # 4 sweeps: (hi, half0), (hi, half1), (lo, half0), (lo, half1).
    # k-tile t of sweep half h: rank r=t//4, i=t%4 -> nodes r*1024+h*512+i*128
    def l1_hi_sweep(half):
        # yk DMAs for each 16-ktile batch issue before the batch's adj
        # stream so they sit at the queue front when the gather lands.
        cco = cch_out[half]
        for b in range(2):
            yks = []
            for t in range(b * 16, (b + 1) * 16):
                r, i = divmod(t, 4)
                ck = (r * 4 + i) * 128
                yk = yswh.tile([128, 512], F16, name="ykh")
                nc.sync.dma_start(yk[:], cco[ck:ck + 128, :])
                yks.append(yk)
            for j, t in enumerate(range(b * 16, (b + 1) * 16)):
                r, i = divmod(t, 4)
                node = r * 1024 + half * 512 + i * 128
                yk = yks[j]
                ah = astream.tile([128, BLK], F16, name="ah1")
                al = astream.tile([128, BLK], F16, name="al1")
                nc.sync.dma_start(ah[:, 0:512], A_h[node:node + 128, 0:512])
                nc.sync.dma_start(ah[:, 512:1024], A_h[node:node + 128, 512:1024])
                nc.sync.dma_start(al[:, 0:512], A_l[node:node + 128, 0:512])
                nc.sync.dma_start(al[:, 512:1024], A_l[node:node + 128, 512:1024])
                first = (half == 0 and t == 0)
                for m in range(4):
                    lt = yk[:, m * 128:(m + 1) * 128]
                    for idx, (rt, n) in enumerate(((ah, 0), (ah, 1), (al, 0), (al, 1))):
                        nc.tensor.matmul(
                            ps[m * 2 + n][:], lt, rt[:, n * 512:(n + 1) * 512],
                            start=(first and idx < 2), stop=False,
                        )

    def l1_lo_sweep(half):
        cco = ccl_out[half]
        for b in range(2):
            yks = []
            for t in range(b * 16, (b + 1) * 16):
                r, i = divmod(t, 4)
                ck = (r * 4 + i) * 128
                yk = yswl.tile([128, 512], F16, name="ykl")
                nc.sync.dma_start(yk[:], cco[ck:ck + 128, :])
                yks.append(yk)
            for j, t in enumerate(range(b * 16, (b + 1) * 16)):
                r, i = divmod(t, 4)
                node = r * 1024 + half * 512 + i * 128
                yk = yks[j]
                ah = astream.tile([128, BLK], F16, name="ah2")
                nc.sync.dma_start(ah[:, 0:512], A_h[node:node + 128, 0:512])
                nc.sync.dma_start(ah[:, 512:1024], A_h[node:node + 128, 512:1024])
                last = (half == 1 and t == 31)
                for m in range(4):
                    lt = yk[:, m * 128:(m + 1) * 128]
                    for n in range(2):
                        nc.tensor.matmul(
                            ps[m * 2 + n][:], lt, ah[:, n * 512:(n + 1) * 512],
                            start=False, stop=last,
                        )

    l1_hi_sweep(0)
    l1_hi_sweep(1)
    l1_lo_sweep(0)
    l1_lo_sweep(1)

    h1t = [acts.tile([128, BLK], F32, name=f"tt_{m}") for m in range(4)]
    h1t16 = [acts.tile([128, BLK], F16, name=f"h1t16_{m}") for m in range(4)]
    for n in range(2):
        for m in range(4):
            nc.scalar.activation(
                h1t16[m][:, n * 512:(n + 1) * 512], ps[m * 2 + n][:],
                AFT.Relu, bias=b1_t[m][:], scale=1.0 / (SCALE * SCALE),
            )
    for n in range(2):
        for m in range(4):
            nc.scalar.activation(
                h1t[m][:, n * 512:(n + 1) * 512], ps[m * 2 + n][:],
                AFT.Relu, bias=b1_t[m][:], scale=1.0 / (SCALE * SCALE),
            )

    # =========== scores + all-expert heads + one-hot select ==============
    # h1-half of hc @ [Wcat16 | Wp32], plus precomputed h0-half, route.
    for m in range(8):
        pt = ps[m]
        for k in range(4):
            nc.tensor.matmul(
                pt[:, 0:OC],
                h1t16[k][:, m * 128:(m + 1) * 128],
                wcat_t[k][:],
                start=(k == 0),
                stop=(k == 3),
            )
        for k in range(4):
            nc.tensor.matmul(
                pt[:, SCOL:SCOL + E],
                h1t[k][:, m * 128:(m + 1) * 128],
                wp_t[k][:],
                start=(k == 0),
                stop=(k == 3),
            )
        allw = small.tile([128, OC], F32, name="allw")
        nc.vector.tensor_tensor(allw[:], pt[:, 0:OC], oa0[m][:], op=ALU.add)
        scf = small.tile([128, E], F32, name="scf")
        nc.vector.tensor_tensor(scf[:], pt[:, SCOL:SCOL + E], sc0[m][:], op=ALU.add)
        sc = scf[:]
        oa = allw[:, 0:OC]
        rmax = small.tile([128, 1], F32, name="rmax")
        nc.vector.tensor_reduce(rmax[:], sc, axis=mybir.AxisListType.X, op=ALU.max)
        # val = (score < max)*1024 + expert_index; first argmax has min val
        val = small.tile([128, E], F32, name="val")
        nc.vector.tensor_scalar(val[:], sc, rmax[:], 1024.0, ALU.is_lt, ALU.mult)
        nc.vector.tensor_tensor(val[:], val[:], iota_t[:], op=ALU.add)
        idxf = small.tile([128, 1], F32, name="idxf")
        nc.vector.tensor_reduce(idxf[:], val[:], axis=mybir.AxisListType.X, op=ALU.min)
        onehot = small.tile([128, E], F32, name="onehot")
        nc.vector.tensor_scalar(onehot[:], val[:], idxf[:], None, ALU.is_equal)
        # masked = out_all * onehot (broadcast over the 32 classes), sum over e
        masked = small.tile([128, E, C], F32, name="masked")
        oa_v = oa.rearrange("p (e c) -> p e c", e=E)
        oh_v = onehot[:, :, None].broadcast_to((128, E, C))
        nc.vector.tensor_tensor(masked[:], oa_v, oh_v, op=ALU.mult)
        out_m = small.tile([128, C], F32, name="out_m")
        mv = masked[:].rearrange("p e c -> p c e")
        nc.vector.tensor_reduce(out_m[:], mv, axis=mybir.AxisListType.X, op=ALU.add)
        nc.sync.dma_start(out[m * 128:(m + 1) * 128, :], out_m[:])


def _build_nc():
    nc = bacc.Bacc("TRN2", target_bir_lowering=False, debug=False,
                   num_devices=NCORES)
    aps = {}
    def inp(name, shape, dt):
        aps[name] = nc.dram_tensor(name, shape, dt, kind="ExternalInput").ap()
    inp("A_h", [N, BLK], F16)
    inp("A_l", [N, BLK], F16)
    inp("F_h", [N, IN], F16)
    inp("F_l", [N, IN], F16)
    inp("W0", [IN, H], F32)
    inp("W1s", [H, H], F32)
    inp("b0", [H, 1], F32)
    inp("b1", [H, 1], F32)
    inp("Wcat16", [2 * H, OC], F16)
    inp("Wp32", [2 * H, E], F32)
    inp("iota7", [128, E], F32)
    aps["out"] = nc.dram_tensor("out", [BLK, C], F32, kind="ExternalOutput").ap()
    for nm in ("cc_h0", "cc_h1", "cc_l0", "cc_l1"):
        aps[nm + "_in"] = nc.dram_tensor(nm + "_in", [512, H], F16).ap()
        aps[nm + "_out"] = nc.dram_tensor(nm + "_out", [8 * 512, H], F16,
                                          addr_space="Shared").ap()
    from contextlib import ExitStack
    with tile.TileContext(nc) as tc, ExitStack() as ctx:
        _kernel_body(ctx, tc, aps)
    nc.compile()
    return nc


def _split16(x):
    h = x.astype(np.float16)
    l = (x - h.astype(np.float32)).astype(np.float16)
    return h, l


def kernel(feature, adj, W0, b0, W1, b1, Wp, Wpp):
    global LAST_RESULTS, _CACHED_NC
    feature = np.ascontiguousarray(np.asarray(feature, dtype=np.float32))
    adj = np.asarray(adj, dtype=np.float32)
    W0 = np.asarray(W0, dtype=np.float32)
    b0 = np.asarray(b0, dtype=np.float32)
    W1 = np.asarray(W1, dtype=np.float32)
    b1 = np.asarray(b1, dtype=np.float32)
    Wp = np.asarray(Wp, dtype=np.float32)
    Wpp = np.asarray(Wpp, dtype=np.float32)

    if _CACHED_NC is None:
        _CACHED_NC = _build_nc()
    nc = _CACHED_NC

    F_h, F_l = _split16(feature)
    Wcat16 = np.ascontiguousarray(
        Wpp.transpose(1, 0, 2).reshape(2 * H, E * C).astype(np.float16))
    iota7 = np.tile(np.arange(E, dtype=np.float32), (128, 1))
    shared = {
        "F_h": F_h, "F_l": F_l,
        "W0": np.ascontiguousarray(W0),
        "W1s": np.ascontiguousarray(W1 * SCALE),
        "b0": b0.reshape(H, 1), "b1": b1.reshape(H, 1),
        "Wcat16": Wcat16, "Wp32": np.ascontiguousarray(Wp), "iota7": iota7,
    }
    in_maps = []
    for c in range(NCORES):
        blk = adj[c * BLK:(c + 1) * BLK, :].T.astype(np.float32) * SCALE
        A_h, A_l = _split16(blk)
        m = dict(shared)
        m["A_h"] = np.ascontiguousarray(A_h)
        m["A_l"] = np.ascontiguousarray(A_l)
        in_maps.append(m)

    trace = os.environ.get("BASS_KERNEL_TRACE", "0") == "1"
    res = run_bass_kernel_spmd(nc, in_maps, list(range(NCORES)), trace=trace)
    LAST_RESULTS = res
    out = np.concatenate([res.results[c]["out"] for c in range(NCORES)], axis=0)
    return out


# revision 16
# speedup vs baseline: 1.0310x; 1.0310x over previous
"""GPPT (GCN + prompt MoE routing) Trainium2 kernel, 8-core SPMD.

Row-shards the N=8192 nodes across 8 NeuronCores (1024 rows each).
Each core holds its block of adj (pre-transposed + scaled on host) and
computes:

  L0:    T^T  = feature^T @ adjT_blk          (3-pass fp16 hi/lo split)
  h0^T   = relu((W0^T @ T^T) * 2^-13 + b0)    (f32r)
  Y1_blk = h0_blk @ (W1*8192)                 (f32r) -> fp16 hi/lo
  4 chunked AllGathers of Y1 (hi/lo x node-halves), first triggered
    before the Y1 phase finishes; expert h0-half precomputed in the
    gather latency window.
  L1:    h1^T = relu((Y1^T @ adjT_blk) * 2^-26 + b1), restructured as
    4 sweeps (hi-half0, hi-half1, lo-half0, lo-half1) so each sweep
    only waits on its own chunked collective.  Collective-dependent
    Y DMAs issue on the ACT hwdge queues so they can't head-of-line
    block the adj stream on the SP queues.
  scores/experts: hc @ [Wp | WppT]            (f32r), one-hot select

Matmuls are grouped by stationary operand (one fp16 lhsT slice feeds
2-4 consecutive matmuls) to enable LDWEIGHTS amortization.

Precision: the expert routing argmax has a 2.5e-7 minimum top-2 score
gap on this input; the two big adj matmuls therefore run as 3-pass
fp16 hi/lo split products (A~Ah+Al, X~Xh+Xl, AX~AhXh+AlXh+AhXl) with
adj scaled by 8192 so the fp16 splits stay in the normal range. This
is fp32-grade (verified: 0 routing flips, ~5e-7 output rel err).
"""

import os
import numpy as np

import concourse.bass as bass
import concourse.bass_utils as _bass_utils
import concourse.mybir as mybir
import concourse.tile as tile
from concourse import bacc
from concourse.bass_utils import run_bass_kernel_spmd


N = 8192
IN = 512
H = 512
C = 32
E = 7
NCORES = 8
BLK = N // NCORES          # 1024 nodes per core
KT = N // 128              # 64 contraction k-tiles over nodes
SCALE = 8192.0             # adj pre-scale (exact power of two)
OC = E * C                 # 224 expert-head output cols (fp16 matmul)
SCOL = 256                 # psum col offset of the fp32 score region
TW = SCOL + E + 1          # 264 cols drained from psum (oa | pad | scores)

F32 = mybir.dt.float32
F16 = mybir.dt.float16

# stashed by kernel() for test harnesses: BassKernelResults of last run
LAST_RESULTS = None
_CACHED_NC = None


def _kernel_body(ctx, tc, aps):
    nc = tc.nc
    AFT = mybir.ActivationFunctionType
    ALU = mybir.AluOpType

    A_h, A_l = aps["A_h"], aps["A_l"]
    F_h, F_l = aps["F_h"], aps["F_l"]
    W0, W1s = aps["W0"], aps["W1s"]
    b0, b1 = aps["b0"], aps["b1"]
    Wcat16 = aps["Wcat16"]      # [2H, 224] = WppT in fp16 (expert heads)
    Wp32 = aps["Wp32"]          # [2H, 7] fp32 prompt/routing weights
    iota7 = aps["iota7"]        # [128, 7] fp32 0..6 per row
    out = aps["out"]
    cch_in = [aps["cc_h0_in"], aps["cc_h1_in"]]
    ccl_in = [aps["cc_l0_in"], aps["cc_l1_in"]]
    cch_out = [aps["cc_h0_out"], aps["cc_h1_out"]]
    ccl_out = [aps["cc_l0_out"], aps["cc_l1_out"]]

    const = ctx.enter_context(tc.tile_pool(name="const", bufs=1))
    acts = ctx.enter_context(tc.tile_pool(name="acts", bufs=1))
    stream = ctx.enter_context(tc.tile_pool(name="stream", bufs=4))
    ypool = ctx.enter_context(tc.tile_pool(name="ypool", bufs=3))
    yswh = ctx.enter_context(tc.tile_pool(name="yswh", bufs=20))
    yswl = ctx.enter_context(tc.tile_pool(name="yswl", bufs=10))
    astream = ctx.enter_context(tc.tile_pool(name="astream", bufs=10))
    small = ctx.enter_context(tc.tile_pool(name="small", bufs=4))
    psum = ctx.enter_context(tc.tile_pool(name="psum", bufs=1, space="PSUM"))

    # ---- 8 PSUM bank accumulators, reused phase to phase ----
    ps = [psum.tile([128, 512], F32, name=f"bank{i}") for i in range(8)]

    # ---- L0 stream tiles ----
    def l0_tiles(k):
        fh = stream.tile([128, IN], F16, name="fh")
        fl = stream.tile([128, IN], F16, name="fl")
        ah = stream.tile([128, BLK], F16, name="ah")
        al = stream.tile([128, BLK], F16, name="al")
        r = slice(k * 128, (k + 1) * 128)
        nc.sync.dma_start(fh[:], F_h[r, :])
        nc.sync.dma_start(fl[:], F_l[r, :])
        nc.sync.dma_start(ah[:, 0:512], A_h[r, 0:512])
        nc.sync.dma_start(ah[:, 512:1024], A_h[r, 512:1024])
        nc.sync.dma_start(al[:, 0:512], A_l[r, 0:512])
        nc.sync.dma_start(al[:, 512:1024], A_l[r, 512:1024])
        return fh, fl, ah, al

    # =========== L0: TT[m,n] = sum_k F[k][:,m].T @ A[k][:,n] (3-pass) =====
    # grouped by stationary: fh_m feeds 4 matmuls, fl_m feeds 2.
    for k in range(KT):
        fh, fl, ah, al = l0_tiles(k)
        for m in range(4):
            lt = fh[:, m * 128:(m + 1) * 128]
            for idx, (rt, n) in enumerate(((ah, 0), (ah, 1), (al, 0), (al, 1))):
                nc.tensor.matmul(
                    ps[m * 2 + n][:], lt, rt[:, n * 512:(n + 1) * 512],
                    start=(k == 0 and idx < 2), stop=False,
                )
        for m in range(4):
            lt = fl[:, m * 128:(m + 1) * 128]
            for n in range(2):
                nc.tensor.matmul(
                    ps[m * 2 + n][:], lt, ah[:, n * 512:(n + 1) * 512],
                    start=False, stop=(k == KT - 1),
                )

    # ---- constants / weights resident in SBUF.  Emitted after the L0
    # loop so they queue behind the first adj k-tiles (startup latency),
    # but arrive long before their first use. ----
    w0_t = []
    w1_t = []
    for k in range(4):
        t = const.tile([128, H], F32, name=f"w0_{k}")
        nc.sync.dma_start(t[:], W0[k * 128:(k + 1) * 128, :])
        w0_t.append(t)
        t = const.tile([128, H], F32, name=f"w1_{k}")
        nc.sync.dma_start(t[:], W1s[k * 128:(k + 1) * 128, :])
        w1_t.append(t)
    wcat_t = []
    wp_t = []
    for k in range(8):
        t = const.tile([128, OC], F16, name=f"wcat_{k}")
        nc.sync.dma_start(t[:], Wcat16[k * 128:(k + 1) * 128, :])
        wcat_t.append(t)
        t = const.tile([128, E], F32, name=f"wp_{k}")
        nc.sync.dma_start(t[:], Wp32[k * 128:(k + 1) * 128, :])
        wp_t.append(t)
    b0_t = []
    b1_t = []
    for m in range(4):
        t = const.tile([128, 1], F32, name=f"b0_{m}")
        nc.sync.dma_start(t[:], b0[m * 128:(m + 1) * 128, :])
        b0_t.append(t)
        t = const.tile([128, 1], F32, name=f"b1_{m}")
        nc.sync.dma_start(t[:], b1[m * 128:(m + 1) * 128, :])
        b1_t.append(t)
    iota_t = const.tile([128, E], F32, name="iota7")
    nc.sync.dma_start(iota_t[:], iota7[:, :])

    # copy TT out of PSUM (raw, still scaled by 8192); node-half n=0
    # first so the first half of the W0 apply can start sooner.
    tt = [acts.tile([128, BLK], F32, name=f"tt_{m}") for m in range(4)]
    for n in range(2):
        for m in range(4):
            nc.vector.tensor_copy(tt[m][:, n * 512:(n + 1) * 512], ps[m * 2 + n][:])

    # h0T[m,n] = relu(2^-13 * sum_k W0[k][:,m].T @ TT[k][:,n] + b0)
    # n-major: the n=0 node-half feeds Y1 chunks 0..3 (= the first
    # AllGather payload), so it runs before the n=1 half.
    h0t = [acts.tile([128, BLK], F32, name=f"h0t_{m}") for m in range(4)]
    h0t16 = [acts.tile([128, BLK], F16, name=f"h0t16_{m}") for m in range(4)]

    def w0_apply(n):
        for m in range(4):
            pt = ps[m * 2 + n]
            for k in range(4):
                nc.tensor.matmul(
                    pt[:],
                    w0_t[k][:, m * 128:(m + 1) * 128],
                    tt[k][:, n * 512:(n + 1) * 512],
                    start=(k == 0),
                    stop=(k == 3),
                )
            nc.scalar.activation(
                h0t[m][:, n * 512:(n + 1) * 512], pt[:],
                AFT.Relu, bias=b0_t[m][:], scale=1.0 / SCALE,
            )
            nc.vector.tensor_copy(
                h0t16[m][:, n * 512:(n + 1) * 512],
                h0t[m][:, n * 512:(n + 1) * 512])

    # =========== Y1s[m] = sum_k h0t[k][:,m].T @ W1s[k]  (node-major) =======
    # fp16 hi/lo to 4 chunked cc buffers; first AllGather fires at m==3.
    def ag(ib, ob):
        nc.gpsimd.collective_compute(
            "AllGather",
            mybir.AluOpType.bypass,
            replica_groups=[list(range(NCORES))],
            ins=[ib.opt()],
            outs=[ob.opt()],
        )

    def y1_chunk(m):
        pt = ps[m]
        for k in range(4):
            nc.tensor.matmul(
                pt[:],
                h0t[k][:, m * 128:(m + 1) * 128],
                w1_t[k][:],
                start=(k == 0),
                stop=(k == 3),
            )
        yh = ypool.tile([128, H], F16, name="yh")
        yl = ypool.tile([128, H], F16, name="yl")
        nc.vector.tensor_copy(yh[:], pt[:])
        nc.vector.tensor_tensor(yl[:], pt[:], yh[:], op=mybir.AluOpType.subtract)
        half, row = divmod(m * 128, 512)
        nc.sync.dma_start(cch_in[half][row:row + 128, :], yh[:])
        nc.sync.dma_start(ccl_in[half][row:row + 128, :], yl[:])

    w0_apply(0)
    for m in range(4):
        y1_chunk(m)
    ag(cch_in[0], cch_out[0])
    w0_apply(1)
    for m in range(4, 8):
        y1_chunk(m)
    ag(cch_in[1], cch_out[1])
    ag(ccl_in[0], ccl_out[0])
    ag(ccl_in[1], ccl_out[1])

    # ---- expert h0-half precompute (fills part of the gather latency) ----
    # oa0[m] = sum_{k=4..7} hc16[k] @ Wcat16[k] (fp16 out_all half)
    # sc0[m] = sum_{k=4..7} hc[k] @ Wp32[k]     (fp32 score half)
    oa0 = [acts.tile([128, OC], F16, name=f"oa0_{m}") for m in range(8)]
    sc0 = [acts.tile([128, E], F32, name=f"sc0_{m}") for m in range(8)]
    for m in range(8):
        pt = ps[m]
        for k in range(4):
            nc.tensor.matmul(
                pt[:, 0:OC],
                h0t16[k][:, m * 128:(m + 1) * 128],
                wcat_t[4 + k][:],
                start=(k == 0),
                stop=(k == 3),
            )
        for k in range(4):
            nc.tensor.matmul(
                pt[:, SCOL:SCOL + E],
                h0t[k][:, m * 128:(m + 1) * 128],
                wp_t[4 + k][:],
                start=(k == 0),
                stop=(k == 3),
            )
        nc.vector.tensor_copy(oa0[m][:], pt[:, 0:OC])
        nc.vector.tensor_copy(sc0[m][:], pt[:, SCOL:SCOL + E])

    # =========== L1: h1T = sum over nodes of Y^T @ adjT (3-pass) ==========
    # 4 sweeps: (hi, half0), (hi, half1), (lo, half0), (lo, half1).
    # k-tile t of sweep half h: rank r=t//4, i=t%4 -> nodes r*1024+h*512+i*128
    def l1_hi_sweep(half):
        # yk DMAs for each 16-ktile batch issue before the batch's adj
        # stream so they sit at the queue front when the gather lands.
        cco = cch_out[half]
        for b in range(2):
            yks = []
            for t in range(b * 16, (b + 1) * 16):
                r, i = divmod(t, 4)
                ck = (r * 4 + i) * 128
                yk = yswh.tile([128, 512], F16, name="ykh")
                nc.sync.dma_start(yk[:], cco[ck:ck + 128, :])
                yks.append(yk)
            for j, t in enumerate(range(b * 16, (b + 1) * 16)):
                r, i = divmod(t, 4)
                node = r * 1024 + half * 512 + i * 128
                yk = yks[j]
                ah = astream.tile([128, BLK], F16, name="ah1")
                al = astream.tile([128, BLK], F16, name="al1")
                nc.sync.dma_start(ah[:, 0:512], A_h[node:node + 128, 0:512])
                nc.sync.dma_start(ah[:, 512:1024], A_h[node:node + 128, 512:1024])
                nc.sync.dma_start(al[:, 0:512], A_l[node:node + 128, 0:512])
                nc.sync.dma_start(al[:, 512:1024], A_l[node:node + 128, 512:1024])
                first = (half == 0 and t == 0)
                for m in range(4):
                    lt = yk[:, m * 128:(m + 1) * 128]
                    for idx, (rt, n) in enumerate(((ah, 0), (ah, 1), (al, 0), (al, 1))):
                        nc.tensor.matmul(
                            ps[m * 2 + n][:], lt, rt[:, n * 512:(n + 1) * 512],
                            start=(first and idx < 2), stop=False,
                        )

    def l1_lo_sweep(half):
        cco = ccl_out[half]
        for b in range(2):
            yks = []
            for t in range(b * 16, (b + 1) * 16):
                r, i = divmod(t, 4)
                ck = (r * 4 + i) * 128
                yk = yswl.tile([128, 512], F16, name="ykl")
                nc.sync.dma_start(yk[:], cco[ck:ck + 128, :])
                yks.append(yk)
            for j, t in enumerate(range(b * 16, (b + 1) * 16)):
                r, i = divmod(t, 4)
                node = r * 1024 + half * 512 + i * 128
                yk = yks[j]
                ah = astream.tile([128, BLK], F16, name="ah2")
                nc.sync.dma_start(ah[:, 0:512], A_h[node:node + 128, 0:512])
                nc.sync.dma_start(ah[:, 512:1024], A_h[node:node + 128, 512:1024])
                last = (half == 1 and t == 31)
                for m in range(4):
                    lt = yk[:, m * 128:(m + 1) * 128]
                    for n in range(2):
                        nc.tensor.matmul(
                            ps[m * 2 + n][:], lt, ah[:, n * 512:(n + 1) * 512],
                            start=False, stop=last,
                        )

    l1_hi_sweep(0)
    l1_hi_sweep(1)
    l1_lo_sweep(0)
    l1_lo_sweep(1)

    h1t = [acts.tile([128, BLK], F32, name=f"tt_{m}") for m in range(4)]
    h1t16 = [acts.tile([128, BLK], F16, name=f"h1t16_{m}") for m in range(4)]
    for n in range(2):
        for m in range(4):
            nc.scalar.activation(
                h1t16[m][:, n * 512:(n + 1) * 512], ps[m * 2 + n][:],
                AFT.Relu, bias=b1_t[m][:], scale=1.0 / (SCALE * SCALE),
            )
    for n in range(2):
        for m in range(4):
            nc.scalar.activation(
                h1t[m][:, n * 512:(n + 1) * 512], ps[m * 2 + n][:],
                AFT.Relu, bias=b1_t[m][:], scale=1.0 / (SCALE * SCALE),
            )

    # =========== scores + all-expert heads + one-hot select ==============
    # h1-half of hc @ [Wcat16 | Wp32], plus precomputed h0-half, route.
    for m in range(8):
        pt = ps[m]
        for k in range(4):
            nc.tensor.matmul(
                pt[:, 0:OC],
                h1t16[k][:, m * 128:(m + 1) * 128],
                wcat_t[k][:],
                start=(k == 0),
                stop=(k == 3),
            )
        for k in range(4):
            nc.tensor.matmul(
                pt[:, SCOL:SCOL + E],
                h1t[k][:, m * 128:(m + 1) * 128],
                wp_t[k][:],
                start=(k == 0),
                stop=(k == 3),
            )
        allw = small.tile([128, OC], F32, name="allw")
        nc.vector.tensor_tensor(allw[:], pt[:, 0:OC], oa0[m][:], op=ALU.add)
        scf = small.tile([128, E], F32, name="scf")
        nc.vector.tensor_tensor(scf[:], pt[:, SCOL:SCOL + E], sc0[m][:], op=ALU.add)
        sc = scf[:]
        oa = allw[:, 0:OC]
        rmax = small.tile([128, 1], F32, name="rmax")
        nc.vector.tensor_reduce(rmax[:], sc, axis=mybir.AxisListType.X, op=ALU.max)
        # val = (score < max)*1024 + expert_index; first argmax has min val
        val = small.tile([128, E], F32, name="val")
        nc.vector.tensor_scalar(val[:], sc, rmax[:], 1024.0, ALU.is_lt, ALU.mult)
        nc.vector.tensor_tensor(val[:], val[:], iota_t[:], op=ALU.add)
        idxf = small.tile([128, 1], F32, name="idxf")
        nc.vector.tensor_reduce(idxf[:], val[:], axis=mybir.AxisListType.X, op=ALU.min)
        onehot = small.tile([128, E], F32, name="onehot")
        nc.vector.tensor_scalar(onehot[:], val[:], idxf[:], None, ALU.is_equal)
        # masked = out_all * onehot (broadcast over the 32 classes), sum over e
        masked = small.tile([128, E, C], F32, name="masked")
        oa_v = oa.rearrange("p (e c) -> p e c", e=E)
        oh_v = onehot[:, :, None].broadcast_to((128, E, C))
        nc.vector.tensor_tensor(masked[:], oa_v, oh_v, op=ALU.mult)
        out_m = small.tile([128, C], F32, name="out_m")
        mv = masked[:].rearrange("p e c -> p c e")
        nc.vector.tensor_reduce(out_m[:], mv, axis=mybir.AxisListType.X, op=ALU.add)
        nc.sync.dma_start(out[m * 128:(m + 1) * 128, :], out_m[:])


def _build_nc():
    nc = bacc.Bacc("TRN2", target_bir_lowering=False, debug=False,
                   num_devices=NCORES)
    aps = {}
    def inp(name, shape, dt):
        aps[name] = nc.dram_tensor(name, shape, dt, kind="ExternalInput").ap()
    inp("A_h", [N, BLK], F16)
    inp("A_l", [N, BLK], F16)
    inp("F_h", [N, IN], F16)
    inp("F_l", [N, IN], F16)
    inp("W0", [IN, H], F32)
    inp("W1s", [H, H], F32)
    inp("b0", [H, 1], F32)
    inp("b1", [H, 1], F32)
    inp("Wcat16", [2 * H, OC], F16)
    inp("Wp32", [2 * H, E], F32)
    inp("iota7", [128, E], F32)
    aps["out"] = nc.dram_tensor("out", [BLK, C], F32, kind="ExternalOutput").ap()
    for nm in ("cc_h0", "cc_h1", "cc_l0", "cc_l1"):
        aps[nm + "_in"] = nc.dram_tensor(nm + "_in", [512, H], F16).ap()
        aps[nm + "_out"] = nc.dram_tensor(nm + "_out", [8 * 512, H], F16,
                                          addr_space="Shared").ap()
    from contextlib import ExitStack
    with tile.TileContext(nc) as tc, ExitStack() as ctx:
        _kernel_body(ctx, tc, aps)
    nc.compile()
    return nc


def _split16(x):
    h = x.astype(np.float16)
    l = (x - h.astype(np.float32)).astype(np.float16)
    return h, l


def kernel(feature, adj, W0, b0, W1, b1, Wp, Wpp):
    global LAST_RESULTS, _CACHED_NC
    feature = np.ascontiguousarray(np.asarray(feature, dtype=np.float32))
    adj = np.asarray(adj, dtype=np.float32)
    W0 = np.asarray(W0, dtype=np.float32)
    b0 = np.asarray(b0, dtype=np.float32)
    W1 = np.asarray(W1, dtype=np.float32)
    b1 = np.asarray(b1, dtype=np.float32)
    Wp = np.asarray(Wp, dtype=np.float32)
    Wpp = np.asarray(Wpp, dtype=np.float32)

    if _CACHED_NC is None:
        _CACHED_NC = _build_nc()
    nc = _CACHED_NC

    F_h, F_l = _split16(feature)
    Wcat16 = np.ascontiguousarray(
        Wpp.transpose(1, 0, 2).reshape(2 * H, E * C).astype(np.float16))
    iota7 = np.tile(np.arange(E, dtype=np.float32), (128, 1))
    shared = {
        "F_h": F_h, "F_l": F_l,
        "W0": np.ascontiguousarray(W0),
        "W1s": np.ascontiguousarray(W1 * SCALE),
        "b0": b0.reshape(H, 1), "b1": b1.reshape(H, 1),
        "Wcat16": Wcat16, "Wp32": np.ascontiguousarray(Wp), "iota7": iota7,
    }
    in_maps = []
    for c in range(NCORES):
        blk = adj[c * BLK:(c + 1) * BLK, :].T.astype(np.float32) * SCALE
        A_h, A_l = _split16(blk)
        m = dict(shared)
        m["A_h"] = np.ascontiguousarray(A_h)
        m["A_l"] = np.ascontiguousarray(A_l)
        in_maps.append(m)

    trace = os.environ.get("BASS_KERNEL_TRACE", "0") == "1"
    res = run_bass_kernel_spmd(nc, in_maps, list(range(NCORES)), trace=trace)
    LAST_RESULTS = res
    out = np.concatenate([res.results[c]["out"] for c in range(NCORES)], axis=0)
    return out


# revision 17
# speedup vs baseline: 1.0387x; 1.0075x over previous
"""GPPT (GCN + prompt MoE routing) Trainium2 kernel, 8-core SPMD.

Row-shards the N=8192 nodes across 8 NeuronCores (1024 rows each).
Each core holds its block of adj (pre-transposed + scaled on host) and
computes:

  L0:    T^T  = feature^T @ adjT_blk          (3-pass fp16 hi/lo split)
  h0^T   = relu((W0^T @ T^T) * 2^-13 + b0)    (fp32, node-half n=0
           first so the first gather payload is ready early)
  Y1_blk = h0_blk @ (W1*8192)                 (fp32) -> fp16 hi/lo
  4 chunked AllGathers of Y1 (hi/lo x node-halves); the first fires
    mid-Y1 so its ~25-40us latency hides under the phase-1 tail, and
    the expert h0-half + routing-score h0-half precompute fill any
    remaining gap.
  L1:    h1^T = relu((Y1^T @ adjT_blk) * 2^-26 + b1), restructured as
    4 sweeps (hi-half0, hi-half1, lo-half0, lo-half1) so each sweep
    only waits on its own chunked collective.  Per sweep, the 16 yk
    DMAs of a batch issue before the batch's adj stream so they sit at
    the queue front when the gather lands; adj tiles use an 10-deep
    ring so the matmul stream can ride through queue hiccups.
  experts: out_all via fp16 matmuls (tolerance ~1e-2 there), routing
    scores via exact-fp32 N=7 matmuls into a disjoint PSUM region,
    h1 activations emitted in both dtypes straight out of PSUM.

Empirical cadence on this part: [128x128]x[128x512] fp16 matmul
back-to-back = ~262ns regardless of LDWEIGHTS presence (weight loads
are fully hidden); fp32 is exactly 2x that.  The kernel therefore
optimizes matmul count / dtype and overlap, not weight-load reuse.

Precision: the expert routing argmax has a 2.5e-7 minimum top-2 score
gap on this input; the two big adj matmuls therefore run as 3-pass
fp16 hi/lo split products (A~Ah+Al, X~Xh+Xl, AX~AhXh+AlXh+AhXl) with
adj scaled by 8192 so the fp16 splits stay in the normal range. This
is fp32-grade (verified: 0 routing flips, ~5e-7 output rel err).
"""

import os
import numpy as np

import concourse.bass as bass
import concourse.mybir as mybir
import concourse.tile as tile
from concourse import bacc
from concourse.bass_utils import run_bass_kernel_spmd


N = 8192
IN = 512
H = 512
C = 32
E = 7
NCORES = 8
BLK = N // NCORES          # 1024 nodes per core
KT = N // 128              # 64 contraction k-tiles over nodes
SCALE = 8192.0             # adj pre-scale (exact power of two)
OC = E * C                 # 224 expert-head output cols (fp16 matmul)
SCOL = 256                 # psum col offset of the fp32 score region
TW = SCOL + E + 1          # 264 cols drained from psum (oa | pad | scores)

F32 = mybir.dt.float32
F16 = mybir.dt.float16

# stashed by kernel() for test harnesses: BassKernelResults of last run
LAST_RESULTS = None
_CACHED_NC = None


def _kernel_body(ctx, tc, aps):
    nc = tc.nc
    AFT = mybir.ActivationFunctionType
    ALU = mybir.AluOpType

    A_h, A_l = aps["A_h"], aps["A_l"]
    F_h, F_l = aps["F_h"], aps["F_l"]
    W0, W1s = aps["W0"], aps["W1s"]
    b0, b1 = aps["b0"], aps["b1"]
    Wcat16 = aps["Wcat16"]      # [2H, 224] = WppT in fp16 (expert heads)
    Wp32 = aps["Wp32"]          # [2H, 7] fp32 prompt/routing weights
    iota7 = aps["iota7"]        # [128, 7] fp32 0..6 per row
    out = aps["out"]
    cch_in = [aps["cc_h0_in"], aps["cc_h1_in"]]
    ccl_in = [aps["cc_l0_in"], aps["cc_l1_in"]]
    cch_out = [aps["cc_h0_out"], aps["cc_h1_out"]]
    ccl_out = [aps["cc_l0_out"], aps["cc_l1_out"]]

    const = ctx.enter_context(tc.tile_pool(name="const", bufs=1))
    acts = ctx.enter_context(tc.tile_pool(name="acts", bufs=1))
    stream = ctx.enter_context(tc.tile_pool(name="stream", bufs=4))
    ypool = ctx.enter_context(tc.tile_pool(name="ypool", bufs=3))
    yswh = ctx.enter_context(tc.tile_pool(name="yswh", bufs=20))
    yswl = ctx.enter_context(tc.tile_pool(name="yswl", bufs=10))
    astream = ctx.enter_context(tc.tile_pool(name="astream", bufs=10))
    small = ctx.enter_context(tc.tile_pool(name="small", bufs=4))
    psum = ctx.enter_context(tc.tile_pool(name="psum", bufs=1, space="PSUM"))

    # ---- 8 PSUM bank accumulators, reused phase to phase ----
    ps = [psum.tile([128, 512], F32, name=f"bank{i}") for i in range(8)]

    # ---- L0 stream tiles ----
    def l0_tiles(k):
        fh = stream.tile([128, IN], F16, name="fh")
        fl = stream.tile([128, IN], F16, name="fl")
        ah = stream.tile([128, BLK], F16, name="ah")
        al = stream.tile([128, BLK], F16, name="al")
        r = slice(k * 128, (k + 1) * 128)
        nc.sync.dma_start(fh[:], F_h[r, :])
        nc.sync.dma_start(fl[:], F_l[r, :])
        nc.sync.dma_start(ah[:, 0:512], A_h[r, 0:512])
        nc.sync.dma_start(ah[:, 512:1024], A_h[r, 512:1024])
        nc.sync.dma_start(al[:, 0:512], A_l[r, 0:512])
        nc.sync.dma_start(al[:, 512:1024], A_l[r, 512:1024])
        return fh, fl, ah, al

    # =========== L0: TT[m,n] = sum_k F[k][:,m].T @ A[k][:,n] (3-pass) =====
    # grouped by stationary: fh_m feeds 4 matmuls, fl_m feeds 2.
    for k in range(KT):
        fh, fl, ah, al = l0_tiles(k)
        for m in range(4):
            lt = fh[:, m * 128:(m + 1) * 128]
            for idx, (rt, n) in enumerate(((ah, 0), (ah, 1), (al, 0), (al, 1))):
                nc.tensor.matmul(
                    ps[m * 2 + n][:], lt, rt[:, n * 512:(n + 1) * 512],
                    start=(k == 0 and idx < 2), stop=False,
                )
        for m in range(4):
            lt = fl[:, m * 128:(m + 1) * 128]
            for n in range(2):
                nc.tensor.matmul(
                    ps[m * 2 + n][:], lt, ah[:, n * 512:(n + 1) * 512],
                    start=False, stop=(k == KT - 1),
                )

    # ---- constants / weights resident in SBUF.  Emitted after the L0
    # loop so they queue behind the first adj k-tiles (startup latency),
    # but arrive long before their first use. ----
    w0_t = []
    w1_t = []
    for k in range(4):
        t = const.tile([128, H], F32, name=f"w0_{k}")
        nc.sync.dma_start(t[:], W0[k * 128:(k + 1) * 128, :])
        w0_t.append(t)
        t = const.tile([128, H], F32, name=f"w1_{k}")
        nc.sync.dma_start(t[:], W1s[k * 128:(k + 1) * 128, :])
        w1_t.append(t)
    wcat_t = []
    wp_t = []
    for k in range(8):
        t = const.tile([128, OC], F16, name=f"wcat_{k}")
        nc.sync.dma_start(t[:], Wcat16[k * 128:(k + 1) * 128, :])
        wcat_t.append(t)
        t = const.tile([128, E], F32, name=f"wp_{k}")
        nc.sync.dma_start(t[:], Wp32[k * 128:(k + 1) * 128, :])
        wp_t.append(t)
    b0_t = []
    b1_t = []
    for m in range(4):
        t = const.tile([128, 1], F32, name=f"b0_{m}")
        nc.sync.dma_start(t[:], b0[m * 128:(m + 1) * 128, :])
        b0_t.append(t)
        t = const.tile([128, 1], F32, name=f"b1_{m}")
        nc.sync.dma_start(t[:], b1[m * 128:(m + 1) * 128, :])
        b1_t.append(t)
    iota_t = const.tile([128, E], F32, name="iota7")
    nc.sync.dma_start(iota_t[:], iota7[:, :])

    # copy TT out of PSUM (raw, still scaled by 8192); node-half n=0
    # first so the first half of the W0 apply can start sooner.
    tt = [acts.tile([128, BLK], F32, name=f"tt_{m}") for m in range(4)]
    for n in range(2):
        for m in range(4):
            nc.vector.tensor_copy(tt[m][:, n * 512:(n + 1) * 512], ps[m * 2 + n][:])

    # h0T[m,n] = relu(2^-13 * sum_k W0[k][:,m].T @ TT[k][:,n] + b0)
    # n-major: the n=0 node-half feeds Y1 chunks 0..3 (= the first
    # AllGather payload), so it runs before the n=1 half.
    h0t = [acts.tile([128, BLK], F32, name=f"h0t_{m}") for m in range(4)]
    h0t16 = [acts.tile([128, BLK], F16, name=f"h0t16_{m}") for m in range(4)]

    def w0_apply(n):
        for m in range(4):
            pt = ps[m * 2 + n]
            for k in range(4):
                nc.tensor.matmul(
                    pt[:],
                    w0_t[k][:, m * 128:(m + 1) * 128],
                    tt[k][:, n * 512:(n + 1) * 512],
                    start=(k == 0),
                    stop=(k == 3),
                )
            nc.scalar.activation(
                h0t[m][:, n * 512:(n + 1) * 512], pt[:],
                AFT.Relu, bias=b0_t[m][:], scale=1.0 / SCALE,
            )
            nc.vector.tensor_copy(
                h0t16[m][:, n * 512:(n + 1) * 512],
                h0t[m][:, n * 512:(n + 1) * 512])

    # =========== Y1s[m] = sum_k h0t[k][:,m].T @ W1s[k]  (node-major) =======
    # fp16 hi/lo to 4 chunked cc buffers; first AllGather fires at m==3.
    def ag(ib, ob):
        nc.gpsimd.collective_compute(
            "AllGather",
            mybir.AluOpType.bypass,
            replica_groups=[list(range(NCORES))],
            ins=[ib.opt()],
            outs=[ob.opt()],
        )

    def y1_chunk(m):
        pt = ps[m]
        for k in range(4):
            nc.tensor.matmul(
                pt[:],
                h0t[k][:, m * 128:(m + 1) * 128],
                w1_t[k][:],
                start=(k == 0),
                stop=(k == 3),
            )
        yh = ypool.tile([128, H], F16, name="yh")
        yl = ypool.tile([128, H], F16, name="yl")
        nc.vector.tensor_copy(yh[:], pt[:])
        nc.vector.tensor_tensor(yl[:], pt[:], yh[:], op=mybir.AluOpType.subtract)
        half, row = divmod(m * 128, 512)
        nc.sync.dma_start(cch_in[half][row:row + 128, :], yh[:])
        nc.sync.dma_start(ccl_in[half][row:row + 128, :], yl[:])

    w0_apply(0)
    for m in range(4):
        y1_chunk(m)
    ag(cch_in[0], cch_out[0])
    w0_apply(1)
    for m in range(4, 8):
        y1_chunk(m)
    ag(cch_in[1], cch_out[1])
    ag(ccl_in[0], ccl_out[0])
    ag(ccl_in[1], ccl_out[1])

    # ---- expert h0-half precompute (fills part of the gather latency) ----
    # oa0[m] = sum_{k=4..7} hc16[k] @ Wcat16[k] (fp16 out_all half)
    # sc0[m] = sum_{k=4..7} hc[k] @ Wp32[k]     (fp32 score half)
    oa0 = [acts.tile([128, OC], F16, name=f"oa0_{m}") for m in range(8)]
    sc0 = [acts.tile([128, E], F32, name=f"sc0_{m}") for m in range(8)]
    for m in range(8):
        pt = ps[m]
        for k in range(4):
            nc.tensor.matmul(
                pt[:, 0:OC],
                h0t16[k][:, m * 128:(m + 1) * 128],
                wcat_t[4 + k][:],
                start=(k == 0),
                stop=(k == 3),
            )
        for k in range(4):
            nc.tensor.matmul(
                pt[:, SCOL:SCOL + E],
                h0t[k][:, m * 128:(m + 1) * 128],
                wp_t[4 + k][:],
                start=(k == 0),
                stop=(k == 3),
            )
        nc.vector.tensor_copy(oa0[m][:], pt[:, 0:OC])
        nc.vector.tensor_copy(sc0[m][:], pt[:, SCOL:SCOL + E])

    # =========== L1: h1T = sum over nodes of Y^T @ adjT (3-pass) ==========
    # 4 sweeps: (hi, half0), (hi, half1), (lo, half0), (lo, half1).
    # k-tile t of sweep half h: rank r=t//4, i=t%4 -> nodes r*1024+h*512+i*128
    def l1_hi_sweep(half):
        # yk DMAs for each 16-ktile batch issue before the batch's adj
        # stream so they sit at the queue front when the gather lands.
        cco = cch_out[half]
        for b in range(2):
            yks = []
            for t in range(b * 16, (b + 1) * 16):
                r, i = divmod(t, 4)
                ck = (r * 4 + i) * 128
                yk = yswh.tile([128, 512], F16, name="ykh")
                nc.sync.dma_start(yk[:], cco[ck:ck + 128, :])
                yks.append(yk)
            for j, t in enumerate(range(b * 16, (b + 1) * 16)):
                r, i = divmod(t, 4)
                node = r * 1024 + half * 512 + i * 128
                yk = yks[j]
                ah = astream.tile([128, BLK], F16, name="ah1")
                al = astream.tile([128, BLK], F16, name="al1")
                nc.sync.dma_start(ah[:, 0:512], A_h[node:node + 128, 0:512])
                nc.sync.dma_start(ah[:, 512:1024], A_h[node:node + 128, 512:1024])
                nc.sync.dma_start(al[:, 0:512], A_l[node:node + 128, 0:512])
                nc.sync.dma_start(al[:, 512:1024], A_l[node:node + 128, 512:1024])
                first = (half == 0 and t == 0)
                for m in range(4):
                    lt = yk[:, m * 128:(m + 1) * 128]
                    for idx, (rt, n) in enumerate(((ah, 0), (ah, 1), (al, 0), (al, 1))):
                        nc.tensor.matmul(
                            ps[m * 2 + n][:], lt, rt[:, n * 512:(n + 1) * 512],
                            start=(first and idx < 2), stop=False,
                        )

    def l1_lo_sweep(half):
        cco = ccl_out[half]
        for b in range(2):
            yks = []
            for t in range(b * 16, (b + 1) * 16):
                r, i = divmod(t, 4)
                ck = (r * 4 + i) * 128
                yk = yswl.tile([128, 512], F16, name="ykl")
                nc.sync.dma_start(yk[:], cco[ck:ck + 128, :])
                yks.append(yk)
            for j, t in enumerate(range(b * 16, (b + 1) * 16)):
                r, i = divmod(t, 4)
                node = r * 1024 + half * 512 + i * 128
                yk = yks[j]
                ah = astream.tile([128, BLK], F16, name="ah2")
                nc.sync.dma_start(ah[:, 0:512], A_h[node:node + 128, 0:512])
                nc.sync.dma_start(ah[:, 512:1024], A_h[node:node + 128, 512:1024])
                last = (half == 1 and t == 31)
                for m in range(4):
                    lt = yk[:, m * 128:(m + 1) * 128]
                    for n in range(2):
                        nc.tensor.matmul(
                            ps[m * 2 + n][:], lt, ah[:, n * 512:(n + 1) * 512],
                            start=False, stop=last,
                        )

    l1_hi_sweep(0)
    l1_hi_sweep(1)
    l1_lo_sweep(0)
    l1_lo_sweep(1)

    h1t = [acts.tile([128, BLK], F32, name=f"tt_{m}") for m in range(4)]
    h1t16 = [acts.tile([128, BLK], F16, name=f"h1t16_{m}") for m in range(4)]
    for n in range(2):
        for m in range(4):
            nc.scalar.activation(
                h1t16[m][:, n * 512:(n + 1) * 512], ps[m * 2 + n][:],
                AFT.Relu, bias=b1_t[m][:], scale=1.0 / (SCALE * SCALE),
            )
    for n in range(2):
        for m in range(4):
            nc.scalar.activation(
                h1t[m][:, n * 512:(n + 1) * 512], ps[m * 2 + n][:],
                AFT.Relu, bias=b1_t[m][:], scale=1.0 / (SCALE * SCALE),
            )

    # =========== scores + all-expert heads + one-hot select ==============
    # h1-half of hc @ [Wcat16 | Wp32], plus precomputed h0-half, route.
    for m in range(8):
        pt = ps[m]
        for k in range(4):
            nc.tensor.matmul(
                pt[:, 0:OC],
                h1t16[k][:, m * 128:(m + 1) * 128],
                wcat_t[k][:],
                start=(k == 0),
                stop=(k == 3),
            )
        for k in range(4):
            nc.tensor.matmul(
                pt[:, SCOL:SCOL + E],
                h1t[k][:, m * 128:(m + 1) * 128],
                wp_t[k][:],
                start=(k == 0),
                stop=(k == 3),
            )
        allw = small.tile([128, OC], F32, name="allw")
        nc.vector.tensor_tensor(allw[:], pt[:, 0:OC], oa0[m][:], op=ALU.add)
        scf = small.tile([128, E], F32, name="scf")
        nc.vector.tensor_tensor(scf[:], pt[:, SCOL:SCOL + E], sc0[m][:], op=ALU.add)
        sc = scf[:]
        oa = allw[:, 0:OC]
        rmax = small.tile([128, 1], F32, name="rmax")
        nc.vector.tensor_reduce(rmax[:], sc, axis=mybir.AxisListType.X, op=ALU.max)
        # val = (score < max)*1024 + expert_index; first argmax has min val
        val = small.tile([128, E], F32, name="val")
        nc.vector.tensor_scalar(val[:], sc, rmax[:], 1024.0, ALU.is_lt, ALU.mult)
        nc.vector.tensor_tensor(val[:], val[:], iota_t[:], op=ALU.add)
        idxf = small.tile([128, 1], F32, name="idxf")
        nc.vector.tensor_reduce(idxf[:], val[:], axis=mybir.AxisListType.X, op=ALU.min)
        onehot = small.tile([128, E], F32, name="onehot")
        nc.vector.tensor_scalar(onehot[:], val[:], idxf[:], None, ALU.is_equal)
        # masked = out_all * onehot (broadcast over the 32 classes), sum over e
        masked = small.tile([128, E, C], F32, name="masked")
        oa_v = oa.rearrange("p (e c) -> p e c", e=E)
        oh_v = onehot[:, :, None].broadcast_to((128, E, C))
        nc.vector.tensor_tensor(masked[:], oa_v, oh_v, op=ALU.mult)
        out_m = small.tile([128, C], F32, name="out_m")
        mv = masked[:].rearrange("p e c -> p c e")
        nc.vector.tensor_reduce(out_m[:], mv, axis=mybir.AxisListType.X, op=ALU.add)
        nc.sync.dma_start(out[m * 128:(m + 1) * 128, :], out_m[:])


def _build_nc():
    nc = bacc.Bacc("TRN2", target_bir_lowering=False, debug=False,
                   num_devices=NCORES)
    aps = {}
    def inp(name, shape, dt):
        aps[name] = nc.dram_tensor(name, shape, dt, kind="ExternalInput").ap()
    inp("A_h", [N, BLK], F16)
    inp("A_l", [N, BLK], F16)
    inp("F_h", [N, IN], F16)
    inp("F_l", [N, IN], F16)
    inp("W0", [IN, H], F32)
    inp("W1s", [H, H], F32)
    inp("b0", [H, 1], F32)
    inp("b1", [H, 1], F32)
    inp("Wcat16", [2 * H, OC], F16)
    inp("Wp32", [2 * H, E], F32)
    inp("iota7", [128, E], F32)
    aps["out"] = nc.dram_tensor("out", [BLK, C], F32, kind="ExternalOutput").ap()
    for nm in ("cc_h0", "cc_h1", "cc_l0", "cc_l1"):
        aps[nm + "_in"] = nc.dram_tensor(nm + "_in", [512, H], F16).ap()
        aps[nm + "_out"] = nc.dram_tensor(nm + "_out", [8 * 512, H], F16,
                                          addr_space="Shared").ap()
    from contextlib import ExitStack
    with tile.TileContext(nc) as tc, ExitStack() as ctx:
        _kernel_body(ctx, tc, aps)
    nc.compile()
    return nc


def _split16(x):
    h = x.astype(np.float16)
    l = (x - h.astype(np.float32)).astype(np.float16)
    return h, l


def kernel(feature, adj, W0, b0, W1, b1, Wp, Wpp):
    global LAST_RESULTS, _CACHED_NC
    feature = np.ascontiguousarray(np.asarray(feature, dtype=np.float32))
    adj = np.asarray(adj, dtype=np.float32)
    W0 = np.asarray(W0, dtype=np.float32)
    b0 = np.asarray(b0, dtype=np.float32)
    W1 = np.asarray(W1, dtype=np.float32)
    b1 = np.asarray(b1, dtype=np.float32)
    Wp = np.asarray(Wp, dtype=np.float32)
    Wpp = np.asarray(Wpp, dtype=np.float32)

    if _CACHED_NC is None:
        _CACHED_NC = _build_nc()
    nc = _CACHED_NC

    F_h, F_l = _split16(feature)
    Wcat16 = np.ascontiguousarray(
        Wpp.transpose(1, 0, 2).reshape(2 * H, E * C).astype(np.float16))
    iota7 = np.tile(np.arange(E, dtype=np.float32), (128, 1))
    shared = {
        "F_h": F_h, "F_l": F_l,
        "W0": np.ascontiguousarray(W0),
        "W1s": np.ascontiguousarray(W1 * SCALE),
        "b0": b0.reshape(H, 1), "b1": b1.reshape(H, 1),
        "Wcat16": Wcat16, "Wp32": np.ascontiguousarray(Wp), "iota7": iota7,
    }
    in_maps = []
    for c in range(NCORES):
        blk = adj[c * BLK:(c + 1) * BLK, :].T.astype(np.float32) * SCALE
        A_h, A_l = _split16(blk)
        m = dict(shared)
        m["A_h"] = np.ascontiguousarray(A_h)
        m["A_l"] = np.ascontiguousarray(A_l)
        in_maps.append(m)

    trace = os.environ.get("BASS_KERNEL_TRACE", "0") == "1"
    res = run_bass_kernel_spmd(nc, in_maps, list(range(NCORES)), trace=trace)
    LAST_RESULTS = res
    out = np.concatenate([res.results[c]["out"] for c in range(NCORES)], axis=0)
    return out


# revision 18
# speedup vs baseline: 1.0473x; 1.0083x over previous
"""GPPT (GCN + prompt MoE routing) Trainium2 kernel, 8-core SPMD.

Row-shards the N=8192 nodes across 8 NeuronCores (1024 rows each).
Each core holds its block of adj (pre-transposed + scaled on host) and
computes:

  L0:    T^T  = feature^T @ adjT_blk          (3-pass fp16 hi/lo split)
  h0^T   = relu((W0^T @ T^T) * 2^-13 + b0)    (fp32, node-half n=0
           first so the first gather payload is ready early)
  Y1_blk = h0_blk @ (W1*8192)                 (fp32) -> fp16 hi/lo
  4 chunked AllGathers of Y1 (hi/lo x node-halves); the first fires
    mid-Y1 so its ~25-40us latency hides under the phase-1 tail, and
    the expert h0-half + routing-score h0-half precompute fill any
    remaining gap.
  L1:    h1^T = relu((Y1^T @ adjT_blk) * 2^-26 + b1), restructured as
    4 sweeps (hi-half0, hi-half1, lo-half0, lo-half1) so each sweep
    only waits on its own chunked collective.  Per sweep, the 16 yk
    DMAs of a batch issue before the batch's adj stream so they sit at
    the queue front when the gather lands; adj tiles use an 10-deep
    ring so the matmul stream can ride through queue hiccups.
  experts: out_all via fp16 matmuls (tolerance ~1e-2 there), routing
    scores via exact-fp32 N=7 matmuls into a disjoint PSUM region,
    h1 activations emitted in both dtypes straight out of PSUM.

Empirical cadence on this part: [128x128]x[128x512] fp16 matmul
back-to-back = ~262ns regardless of LDWEIGHTS presence (weight loads
are fully hidden); fp32 is exactly 2x that.  The kernel therefore
optimizes matmul count / dtype and overlap, not weight-load reuse.

Precision: the expert routing argmax has a 2.5e-7 minimum top-2 score
gap on this input; the two big adj matmuls therefore run as 3-pass
fp16 hi/lo split products (A~Ah+Al, X~Xh+Xl, AX~AhXh+AlXh+AhXl) with
adj scaled by 8192 so the fp16 splits stay in the normal range. This
is fp32-grade (verified: 0 routing flips, ~5e-7 output rel err).
"""

import os
import numpy as np

import concourse.bass as bass
import concourse.mybir as mybir
import concourse.tile as tile
from concourse import bacc
from concourse.bass_utils import run_bass_kernel_spmd


N = 8192
IN = 512
H = 512
C = 32
E = 7
NCORES = 8
BLK = N // NCORES          # 1024 nodes per core
KT = N // 128              # 64 contraction k-tiles over nodes
SCALE = 8192.0             # adj pre-scale (exact power of two)
OC = E * C                 # 224 expert-head output cols (fp16 matmul)
SCOL = 256                 # psum col offset of the fp32 score region
TW = SCOL + E + 1          # 264 cols drained from psum (oa | pad | scores)

F32 = mybir.dt.float32
F16 = mybir.dt.float16

# stashed by kernel() for test harnesses: BassKernelResults of last run
LAST_RESULTS = None
_CACHED_NC = None


def _kernel_body(ctx, tc, aps):
    nc = tc.nc
    AFT = mybir.ActivationFunctionType
    ALU = mybir.AluOpType

    A_h, A_l = aps["A_h"], aps["A_l"]
    F_h, F_l = aps["F_h"], aps["F_l"]
    W0, W1s = aps["W0"], aps["W1s"]
    b0, b1 = aps["b0"], aps["b1"]
    Wcat16 = aps["Wcat16"]      # [2H, 224] = WppT in fp16 (expert heads)
    Wp32 = aps["Wp32"]          # [2H, 7] fp32 prompt/routing weights
    iota7 = aps["iota7"]        # [128, 7] fp32 0..6 per row
    out = aps["out"]
    cch_in = [aps["cc_h0_in"], aps["cc_h1_in"]]
    ccl_in = [aps["cc_l0_in"], aps["cc_l1_in"]]
    cch_out = [aps["cc_h0_out"], aps["cc_h1_out"]]
    ccl_out = [aps["cc_l0_out"], aps["cc_l1_out"]]

    const = ctx.enter_context(tc.tile_pool(name="const", bufs=1))
    acts = ctx.enter_context(tc.tile_pool(name="acts", bufs=1))
    stream = ctx.enter_context(tc.tile_pool(name="stream", bufs=4))
    ypool = ctx.enter_context(tc.tile_pool(name="ypool", bufs=3))
    yswh = ctx.enter_context(tc.tile_pool(name="yswh", bufs=20))
    yswl = ctx.enter_context(tc.tile_pool(name="yswl", bufs=10))
    astream = ctx.enter_context(tc.tile_pool(name="astream", bufs=10))
    small = ctx.enter_context(tc.tile_pool(name="small", bufs=4))
    psum = ctx.enter_context(tc.tile_pool(name="psum", bufs=1, space="PSUM"))

    # ---- 8 PSUM bank accumulators, reused phase to phase ----
    ps = [psum.tile([128, 512], F32, name=f"bank{i}") for i in range(8)]

    # ---- L0 stream tiles ----
    def l0_tiles(k):
        fh = stream.tile([128, IN], F16, name="fh")
        fl = stream.tile([128, IN], F16, name="fl")
        ah = stream.tile([128, BLK], F16, name="ah")
        al = stream.tile([128, BLK], F16, name="al")
        r = slice(k * 128, (k + 1) * 128)
        nc.sync.dma_start(fh[:], F_h[r, :])
        nc.sync.dma_start(fl[:], F_l[r, :])
        nc.sync.dma_start(ah[:, 0:512], A_h[r, 0:512])
        nc.sync.dma_start(ah[:, 512:1024], A_h[r, 512:1024])
        nc.sync.dma_start(al[:, 0:512], A_l[r, 0:512])
        nc.sync.dma_start(al[:, 512:1024], A_l[r, 512:1024])
        return fh, fl, ah, al

    # =========== L0: TT[m,n] = sum_k F[k][:,m].T @ A[k][:,n] (3-pass) =====
    # grouped by stationary: fh_m feeds 4 matmuls, fl_m feeds 2.
    for k in range(KT):
        fh, fl, ah, al = l0_tiles(k)
        for m in range(4):
            lt = fh[:, m * 128:(m + 1) * 128]
            for idx, (rt, n) in enumerate(((ah, 0), (ah, 1), (al, 0), (al, 1))):
                nc.tensor.matmul(
                    ps[m * 2 + n][:], lt, rt[:, n * 512:(n + 1) * 512],
                    start=(k == 0 and idx < 2), stop=False,
                )
        for m in range(4):
            lt = fl[:, m * 128:(m + 1) * 128]
            for n in range(2):
                nc.tensor.matmul(
                    ps[m * 2 + n][:], lt, ah[:, n * 512:(n + 1) * 512],
                    start=False, stop=(k == KT - 1),
                )

    # ---- constants / weights resident in SBUF.  Emitted after the L0
    # loop so they queue behind the first adj k-tiles (startup latency),
    # but arrive long before their first use. ----
    w0_t = []
    w1_t = []
    for k in range(4):
        t = const.tile([128, H], F32, name=f"w0_{k}")
        nc.sync.dma_start(t[:], W0[k * 128:(k + 1) * 128, :])
        w0_t.append(t)
        t = const.tile([128, H], F32, name=f"w1_{k}")
        nc.sync.dma_start(t[:], W1s[k * 128:(k + 1) * 128, :])
        w1_t.append(t)
    wcat_t = []
    wp_t = []
    for k in range(8):
        t = const.tile([128, OC], F16, name=f"wcat_{k}")
        nc.sync.dma_start(t[:], Wcat16[k * 128:(k + 1) * 128, :])
        wcat_t.append(t)
        t = const.tile([128, E], F32, name=f"wp_{k}")
        nc.sync.dma_start(t[:], Wp32[k * 128:(k + 1) * 128, :])
        wp_t.append(t)
    b0_t = []
    b1_t = []
    for m in range(4):
        t = const.tile([128, 1], F32, name=f"b0_{m}")
        nc.sync.dma_start(t[:], b0[m * 128:(m + 1) * 128, :])
        b0_t.append(t)
        t = const.tile([128, 1], F32, name=f"b1_{m}")
        nc.sync.dma_start(t[:], b1[m * 128:(m + 1) * 128, :])
        b1_t.append(t)
    iota_t = const.tile([128, E], F32, name="iota7")
    nc.sync.dma_start(iota_t[:], iota7[:, :])

    # copy TT out of PSUM (raw, still scaled by 8192); node-half n=0
    # first so the first half of the W0 apply can start sooner.
    tt = [acts.tile([128, BLK], F32, name=f"tt_{m}") for m in range(4)]
    for n in range(2):
        for m in range(4):
            nc.vector.tensor_copy(tt[m][:, n * 512:(n + 1) * 512], ps[m * 2 + n][:])

    # h0T[m,n] = relu(2^-13 * sum_k W0[k][:,m].T @ TT[k][:,n] + b0)
    # n-major: the n=0 node-half feeds Y1 chunks 0..3 (= the first
    # AllGather payload), so it runs before the n=1 half.
    h0t = [acts.tile([128, BLK], F32, name=f"h0t_{m}") for m in range(4)]
    h0t16 = [acts.tile([128, BLK], F16, name=f"h0t16_{m}") for m in range(4)]

    def w0_apply(n):
        for m in range(4):
            pt = ps[m * 2 + n]
            for k in range(4):
                nc.tensor.matmul(
                    pt[:],
                    w0_t[k][:, m * 128:(m + 1) * 128],
                    tt[k][:, n * 512:(n + 1) * 512],
                    start=(k == 0),
                    stop=(k == 3),
                )
            nc.scalar.activation(
                h0t[m][:, n * 512:(n + 1) * 512], pt[:],
                AFT.Relu, bias=b0_t[m][:], scale=1.0 / SCALE,
            )
            nc.vector.tensor_copy(
                h0t16[m][:, n * 512:(n + 1) * 512],
                h0t[m][:, n * 512:(n + 1) * 512])

    # =========== Y1s[m] = sum_k h0t[k][:,m].T @ W1s[k]  (node-major) =======
    # fp16 hi/lo to 4 chunked cc buffers; first AllGather fires at m==3.
    def ag(ib, ob):
        nc.gpsimd.collective_compute(
            "AllGather",
            mybir.AluOpType.bypass,
            replica_groups=[list(range(NCORES))],
            ins=[ib.opt()],
            outs=[ob.opt()],
        )

    def y1_chunk(m):
        pt = ps[m]
        for k in range(4):
            nc.tensor.matmul(
                pt[:],
                h0t[k][:, m * 128:(m + 1) * 128],
                w1_t[k][:],
                start=(k == 0),
                stop=(k == 3),
            )
        yh = ypool.tile([128, H], F16, name="yh")
        yl = ypool.tile([128, H], F16, name="yl")
        nc.vector.tensor_copy(yh[:], pt[:])
        nc.vector.tensor_tensor(yl[:], pt[:], yh[:], op=mybir.AluOpType.subtract)
        half, row = divmod(m * 128, 512)
        nc.sync.dma_start(cch_in[half][row:row + 128, :], yh[:])
        nc.sync.dma_start(ccl_in[half][row:row + 128, :], yl[:])

    w0_apply(0)
    for m in range(4):
        y1_chunk(m)
    ag(cch_in[0], cch_out[0])
    w0_apply(1)
    for m in range(4, 8):
        y1_chunk(m)
    ag(cch_in[1], cch_out[1])
    ag(ccl_in[0], ccl_out[0])
    ag(ccl_in[1], ccl_out[1])

    # ---- expert h0-half precompute (fills part of the gather latency) ----
    # oa0[m] = sum_{k=4..7} hc16[k] @ Wcat16[k] (fp16 out_all half)
    # sc0[m] = sum_{k=4..7} hc[k] @ Wp32[k]     (fp32 score half)
    oa0 = [acts.tile([128, OC], F16, name=f"oa0_{m}") for m in range(8)]
    sc0 = [acts.tile([128, E], F32, name=f"sc0_{m}") for m in range(8)]
    for m in range(8):
        pt = ps[m]
        for k in range(4):
            nc.tensor.matmul(
                pt[:, 0:OC],
                h0t16[k][:, m * 128:(m + 1) * 128],
                wcat_t[4 + k][:],
                start=(k == 0),
                stop=(k == 3),
            )
        for k in range(4):
            nc.tensor.matmul(
                pt[:, SCOL:SCOL + E],
                h0t[k][:, m * 128:(m + 1) * 128],
                wp_t[4 + k][:],
                start=(k == 0),
                stop=(k == 3),
            )
        nc.vector.tensor_copy(oa0[m][:], pt[:, 0:OC])
        nc.vector.tensor_copy(sc0[m][:], pt[:, SCOL:SCOL + E])

    # =========== L1: h1T = sum over nodes of Y^T @ adjT (3-pass) ==========
    # 4 sweeps: (hi, half0), (hi, half1), (lo, half0), (lo, half1).
    # k-tile t of sweep half h: rank r=t//4, i=t%4 -> nodes r*1024+h*512+i*128
    def l1_hi_sweep(half):
        # yk DMAs for each 16-ktile batch issue before the batch's adj
        # stream so they sit at the queue front when the gather lands.
        cco = cch_out[half]
        for b in range(2):
            yks = []
            for t in range(b * 16, (b + 1) * 16):
                r, i = divmod(t, 4)
                ck = (r * 4 + i) * 128
                yk = yswh.tile([128, 512], F16, name="ykh")
                nc.sync.dma_start(yk[:], cco[ck:ck + 128, :])
                yks.append(yk)
            for j, t in enumerate(range(b * 16, (b + 1) * 16)):
                r, i = divmod(t, 4)
                node = r * 1024 + half * 512 + i * 128
                yk = yks[j]
                ah = astream.tile([128, BLK], F16, name="ah1")
                al = astream.tile([128, BLK], F16, name="al1")
                nc.sync.dma_start(ah[:, 0:512], A_h[node:node + 128, 0:512])
                nc.sync.dma_start(ah[:, 512:1024], A_h[node:node + 128, 512:1024])
                nc.sync.dma_start(al[:, 0:512], A_l[node:node + 128, 0:512])
                nc.sync.dma_start(al[:, 512:1024], A_l[node:node + 128, 512:1024])
                first = (half == 0 and t == 0)
                for m in range(4):
                    lt = yk[:, m * 128:(m + 1) * 128]
                    for idx, (rt, n) in enumerate(((ah, 0), (ah, 1), (al, 0), (al, 1))):
                        nc.tensor.matmul(
                            ps[m * 2 + n][:], lt, rt[:, n * 512:(n + 1) * 512],
                            start=(first and idx < 2), stop=False,
                        )

    def l1_lo_sweep(half):
        cco = ccl_out[half]
        for b in range(2):
            yks = []
            for t in range(b * 16, (b + 1) * 16):
                r, i = divmod(t, 4)
                ck = (r * 4 + i) * 128
                yk = yswl.tile([128, 512], F16, name="ykl")
                nc.sync.dma_start(yk[:], cco[ck:ck + 128, :])
                yks.append(yk)
            for j, t in enumerate(range(b * 16, (b + 1) * 16)):
                r, i = divmod(t, 4)
                node = r * 1024 + half * 512 + i * 128
                yk = yks[j]
                ah = astream.tile([128, BLK], F16, name="ah2")
                nc.sync.dma_start(ah[:, 0:512], A_h[node:node + 128, 0:512])
                nc.sync.dma_start(ah[:, 512:1024], A_h[node:node + 128, 512:1024])
                last = (half == 1 and t == 31)
                for m in range(4):
                    lt = yk[:, m * 128:(m + 1) * 128]
                    for n in range(2):
                        nc.tensor.matmul(
                            ps[m * 2 + n][:], lt, ah[:, n * 512:(n + 1) * 512],
                            start=False, stop=last,
                        )

    l1_hi_sweep(0)
    l1_hi_sweep(1)
    l1_lo_sweep(0)
    l1_lo_sweep(1)

    h1t = [acts.tile([128, BLK], F32, name=f"tt_{m}") for m in range(4)]
    h1t16 = [[acts.tile([128, 512], F16, name=f"h1t16_{n}_{m}")
              for m in range(4)] for n in range(2)]
    for n in range(2):
        for m in range(4):
            nc.scalar.activation(
                h1t16[n][m][:], ps[m * 2 + n][:],
                AFT.Relu, bias=b1_t[m][:], scale=1.0 / (SCALE * SCALE),
            )
    for n in range(2):
        for m in range(4):
            nc.scalar.activation(
                h1t[m][:, n * 512:(n + 1) * 512], ps[m * 2 + n][:],
                AFT.Relu, bias=b1_t[m][:], scale=1.0 / (SCALE * SCALE),
            )

    # =========== scores + all-expert heads + one-hot select ==============
    # h1-half of hc @ [Wcat16 | Wp32], plus precomputed h0-half, route.
    for m in range(8):
        pt = ps[m]
        for k in range(4):
            nc.tensor.matmul(
                pt[:, 0:OC],
                h1t16[m // 4][k][:, (m % 4) * 128:(m % 4 + 1) * 128],
                wcat_t[k][:],
                start=(k == 0),
                stop=(k == 3),
            )
        for k in range(4):
            nc.tensor.matmul(
                pt[:, SCOL:SCOL + E],
                h1t[k][:, m * 128:(m + 1) * 128],
                wp_t[k][:],
                start=(k == 0),
                stop=(k == 3),
            )
        allw = small.tile([128, OC], F32, name="allw")
        nc.vector.tensor_tensor(allw[:], pt[:, 0:OC], oa0[m][:], op=ALU.add)
        scf = small.tile([128, E], F32, name="scf")
        nc.vector.tensor_tensor(scf[:], pt[:, SCOL:SCOL + E], sc0[m][:], op=ALU.add)
        sc = scf[:]
        oa = allw[:, 0:OC]
        rmax = small.tile([128, 1], F32, name="rmax")
        nc.vector.tensor_reduce(rmax[:], sc, axis=mybir.AxisListType.X, op=ALU.max)
        # val = (score < max)*1024 + expert_index; first argmax has min val
        val = small.tile([128, E], F32, name="val")
        nc.vector.tensor_scalar(val[:], sc, rmax[:], 1024.0, ALU.is_lt, ALU.mult)
        nc.vector.tensor_tensor(val[:], val[:], iota_t[:], op=ALU.add)
        idxf = small.tile([128, 1], F32, name="idxf")
        nc.vector.tensor_reduce(idxf[:], val[:], axis=mybir.AxisListType.X, op=ALU.min)
        onehot = small.tile([128, E], F32, name="onehot")
        nc.vector.tensor_scalar(onehot[:], val[:], idxf[:], None, ALU.is_equal)
        # masked = out_all * onehot (broadcast over the 32 classes), sum over e
        masked = small.tile([128, E, C], F32, name="masked")
        oa_v = oa.rearrange("p (e c) -> p e c", e=E)
        oh_v = onehot[:, :, None].broadcast_to((128, E, C))
        nc.vector.tensor_tensor(masked[:], oa_v, oh_v, op=ALU.mult)
        out_m = small.tile([128, C], F32, name="out_m")
        mv = masked[:].rearrange("p e c -> p c e")
        nc.vector.tensor_reduce(out_m[:], mv, axis=mybir.AxisListType.X, op=ALU.add)
        nc.scalar.dma_start(out[m * 128:(m + 1) * 128, :], out_m[:])


def _build_nc():
    nc = bacc.Bacc("TRN2", target_bir_lowering=False, debug=False,
                   num_devices=NCORES)
    aps = {}
    def inp(name, shape, dt):
        aps[name] = nc.dram_tensor(name, shape, dt, kind="ExternalInput").ap()
    inp("A_h", [N, BLK], F16)
    inp("A_l", [N, BLK], F16)
    inp("F_h", [N, IN], F16)
    inp("F_l", [N, IN], F16)
    inp("W0", [IN, H], F32)
    inp("W1s", [H, H], F32)
    inp("b0", [H, 1], F32)
    inp("b1", [H, 1], F32)
    inp("Wcat16", [2 * H, OC], F16)
    inp("Wp32", [2 * H, E], F32)
    inp("iota7", [128, E], F32)
    aps["out"] = nc.dram_tensor("out", [BLK, C], F32, kind="ExternalOutput").ap()
    for nm in ("cc_h0", "cc_h1", "cc_l0", "cc_l1"):
        aps[nm + "_in"] = nc.dram_tensor(nm + "_in", [512, H], F16).ap()
        aps[nm + "_out"] = nc.dram_tensor(nm + "_out", [8 * 512, H], F16,
                                          addr_space="Shared").ap()
    from contextlib import ExitStack
    with tile.TileContext(nc) as tc, ExitStack() as ctx:
        _kernel_body(ctx, tc, aps)
    nc.compile()
    return nc


def _split16(x):
    h = x.astype(np.float16)
    l = (x - h.astype(np.float32)).astype(np.float16)
    return h, l


def kernel(feature, adj, W0, b0, W1, b1, Wp, Wpp):
    global LAST_RESULTS, _CACHED_NC
    feature = np.ascontiguousarray(np.asarray(feature, dtype=np.float32))
    adj = np.asarray(adj, dtype=np.float32)
    W0 = np.asarray(W0, dtype=np.float32)
    b0 = np.asarray(b0, dtype=np.float32)
    W1 = np.asarray(W1, dtype=np.float32)
    b1 = np.asarray(b1, dtype=np.float32)
    Wp = np.asarray(Wp, dtype=np.float32)
    Wpp = np.asarray(Wpp, dtype=np.float32)

    if _CACHED_NC is None:
        _CACHED_NC = _build_nc()
    nc = _CACHED_NC

    F_h, F_l = _split16(feature)
    Wcat16 = np.ascontiguousarray(
        Wpp.transpose(1, 0, 2).reshape(2 * H, E * C).astype(np.float16))
    iota7 = np.tile(np.arange(E, dtype=np.float32), (128, 1))
    shared = {
        "F_h": F_h, "F_l": F_l,
        "W0": np.ascontiguousarray(W0),
        "W1s": np.ascontiguousarray(W1 * SCALE),
        "b0": b0.reshape(H, 1), "b1": b1.reshape(H, 1),
        "Wcat16": Wcat16, "Wp32": np.ascontiguousarray(Wp), "iota7": iota7,
    }
    in_maps = []
    for c in range(NCORES):
        blk = adj[c * BLK:(c + 1) * BLK, :].T.astype(np.float32) * SCALE
        A_h, A_l = _split16(blk)
        m = dict(shared)
        m["A_h"] = np.ascontiguousarray(A_h)
        m["A_l"] = np.ascontiguousarray(A_l)
        in_maps.append(m)

    trace = os.environ.get("BASS_KERNEL_TRACE", "0") == "1"
    res = run_bass_kernel_spmd(nc, in_maps, list(range(NCORES)), trace=trace)
    LAST_RESULTS = res
    out = np.concatenate([res.results[c]["out"] for c in range(NCORES)], axis=0)
    return out
